# revision 26
# baseline (speedup 1.0000x reference)
"""Trainium2 Bass kernel for nn_HadamardTransform: out = value @ (weight + permutation).

Data-parallel over the 8192 token rows across 8 NeuronCores (1024 rows/core).
Everything runs in the transposed frame:  o[n, m] = sum_k (H+P)[k,n] vT[k,m]
with H symmetric Sylvester (scaled 1/64) and P a one-hot permutation, so
o = H vT + vT[src, :] where src[n] = argmax_k P[k, n].

Structured path:
  H_4096 = H_8 (x) H_512  (Kronecker, i = i1*512 + i0).
  - PE: per 512-block i1, u_{i1} = (H_512/64) v_{i1}  (bf16 matmuls, fp32 PSUM,
    two 512-col groups share a 2-bank PSUM tile -> one wide Act evacuation)
  - Act: PSUM -> SBUF bf16 evacuation
  - DVE: 3 radix-2 FWHT butterfly stages across the 8 blocks (bf16, all-SBUF,
    [128, 4, 512]-tile ops; finer or coarser granularity both measured slower
    on this part - per-op overhead ~1.5us, strided mega-ops pathological)
  - Permutation term vT[src]: the row reorder is applied host-side as input
    prep (vP input); the add runs on device (DVE + a few on GpSimd).
    On-device indirect-DMA gather was measured 4.5x slower: its traffic
    serializes through the single SWDGE queue (~22 GB/s).
  - All DRAM I/O uses HOST-PRE-TILED layouts (16-32KB contiguous runs per
    partition): descriptor-count-bound DMA measured ~76 GB/s with 1KB runs;
    big runs are bandwidth-bound (24MB I/O in 25.6us/iter in isolation).
  - Inputs on the SP HWDGE queue, outputs on the Act HWDGE queue.
bf16 is exact for H/64 and the butterflies; value rounding gives ~7e-3
relative error vs the 1e-2 gate.
"""

import sys

sys.path.insert(0, "/opt/trn_rl_repo")

import numpy as np

import concourse.bacc as bacc
import concourse.bass as bass
import concourse.mybir as mybir
import concourse.tile as tile
from concourse.bass_utils import run_bass_kernel_spmd

ROWS = 8192
N = 4096
N_CORES = 8
MPC = ROWS // N_CORES  # 1024 token rows per core
KT = N // 128  # 32 k-tiles
NB = N // 128  # 32 n-blocks
MC = MPC // 512  # legacy (dense path m-chunks)

BF16 = mybir.dt.np(mybir.dt.bfloat16)

_cache = {}


# ---------------- structured (Hadamard) path ----------------

B = 512          # PE transform block size
KS = B // 128    # 4 k-subtiles per block
I1 = N // B      # 8 blocks -> 3 DVE butterfly stages
J2S = B // 128   # 4 output 128-row subblocks per block
MH = 512         # m processed in halves
NH = MPC // MH   # 2 halves
N_POOL_ADD = 0   # GpSimd tensor ops measured ~14us launch each -> keep adds on DVE
UNROLL = 2       # reps per For_i iteration (u2 measured best head-to-head)


def _hadamard_pm1(n):
    idx = np.arange(n, dtype=np.int64)
    m = idx[:, None] & idx[None, :]
    pop = np.zeros_like(m)
    for _ in range(int(np.log2(n))):
        pop += m & 1
        m >>= 1
    return np.where(pop % 2 == 0, 1.0, -1.0).astype(np.float32)


def check_structure(weight, permutation):
    """weight must be the scaled Sylvester Hadamard, permutation one-hot."""
    H = _hadamard_pm1(N) / np.sqrt(np.float32(N))
    if not np.array_equal(weight, H):
        return None
    src = np.argmax(permutation, axis=0).astype(np.int32)
    ok = (
        permutation[src, np.arange(N)].min() == 1.0
        and permutation.sum() == N
        and np.abs(permutation).sum() == N
    )
    return src if ok else None


def build_hadamard(reps=1, hw_loop=False):
    nc = bacc.Bacc("TRN2", target_bir_lowering=False)
    # host-pre-tiled layouts: per (partition, half) runs are contiguous 32KB
    vT = nc.dram_tensor("vT", (128, NH, KT, MH), mybir.dt.bfloat16, kind="ExternalInput")
    vP = nc.dram_tensor("vP", (128, NH, NB, MH), mybir.dt.bfloat16, kind="ExternalInput")
    hb = nc.dram_tensor("hb", (B, B), mybir.dt.bfloat16, kind="ExternalInput")
    # output grouped as (g, j2): block j1 = j2*4 + g, pairs share one DMA
    o = nc.dram_tensor("o", (128, NH, 4, 2, J2S, MH), mybir.dt.bfloat16, kind="ExternalOutput")

    add, sub = mybir.AluOpType.add, mybir.AluOpType.subtract

    with tile.TileContext(nc) as tc:
        with (
            tc.tile_pool(name="hbp", bufs=1) as hb_pool,
            tc.tile_pool(name="vt", bufs=2) as vt_pool,
            tc.tile_pool(name="vp", bufs=1) as vp_pool,
            tc.tile_pool(name="ps", bufs=2, space="PSUM") as ps_pool,
            tc.tile_pool(name="u", bufs=1) as u_pool,
            tc.tile_pool(name="b", bufs=1) as b_pool,
        ):
            # H_512/64 as lhsT panels: hbt[p, ks, j] = hb[ks*128+p, j]
            hbt = hb_pool.tile([128, KS, B], mybir.dt.bfloat16, tag="hbt")
            nc.sync.dma_start(
                out=hbt, in_=hb[:, :].rearrange("(ks p) j -> p ks j", p=128)
            )

            if hw_loop and reps > UNROLL:
                assert reps % UNROLL == 0
                loop_cm = tc.For_i(0, reps // UNROLL)
                loop_cm.__enter__()
                rep_range = range(UNROLL)
            else:
                loop_cm = None
                rep_range = range(reps)

            FJ = J2S * MH  # 2048: flattened (j2s, m) per block
            for rep in rep_range:
                for h in range(NH):
                    # 4MB input chunks, 32KB contiguous per partition
                    vts = vt_pool.tile([128, KT, MH], mybir.dt.bfloat16, tag="vts")
                    nc.sync.dma_start(out=vts, in_=vT[:, h, :, :])
                    vps = vp_pool.tile([128, NB, MH], mybir.dt.bfloat16, tag="vps")
                    nc.sync.dma_start(out=vps, in_=vP[:, h, :, :])

                    # PE: u_{i1} = (H/64) v_{i1}, evacuated into stage-packed
                    # tiles: E holds even i1 (slot k: b1=k>>1, b2=k&1),
                    # O holds odd i1 at the matching slot.
                    E = u_pool.tile([128, 4, FJ], mybir.dt.bfloat16, tag="E")
                    O = u_pool.tile([128, 4, FJ], mybir.dt.bfloat16, tag="O")
                    for i1 in range(I1):
                        b0, b1, b2 = i1 & 1, (i1 >> 1) & 1, (i1 >> 2) & 1
                        dst = O if b0 else E
                        k = b1 * 2 + b2
                        for jp in range(J2S // 2):
                            ps = ps_pool.tile([128, 2 * MH], mybir.dt.float32, tag="ps")
                            for half in range(2):
                                j2s = 2 * jp + half
                                for ks in range(KS):
                                    nc.tensor.matmul(
                                        out=ps[:, half * MH : (half + 1) * MH],
                                        lhsT=hbt[:, ks, j2s * 128 : (j2s + 1) * 128],
                                        rhs=vts[:, i1 * KS + ks, :],
                                        start=(ks == 0),
                                        stop=(ks == KS - 1),
                                    )
                            nc.scalar.copy(
                                out=dst[:, k, jp * 2 * MH : (jp + 1) * 2 * MH],
                                in_=ps[:, :],
                            )

                    # DVE FWHT, merged ops (full-tile / contiguous halves only):
                    # s1 (bit0): S = E + O (j0=0), E <- E - O in place (j0=1)
                    S = b_pool.tile([128, 4, FJ], mybir.dt.bfloat16, tag="S")
                    nc.vector.tensor_tensor(out=S, in0=E, in1=O, op=add)
                    nc.vector.tensor_tensor(out=E, in0=E, in1=O, op=sub)
                    # s2 (bit1 = high slot bit): halves; scratch into O and S
                    # O[:,0:2]=j0=0,j1m=0  O[:,2:4]=j0=0,j1m=1
                    nc.vector.tensor_tensor(out=O[:, 0:2, :], in0=S[:, 0:2, :], in1=S[:, 2:4, :], op=add)
                    nc.vector.tensor_tensor(out=O[:, 2:4, :], in0=S[:, 0:2, :], in1=S[:, 2:4, :], op=sub)
                    # S[:,0:2]=j0=1,j1m=0  S[:,2:4]=j0=1,j1m=1  (reads E=D)
                    nc.vector.tensor_tensor(out=S[:, 0:2, :], in0=E[:, 0:2, :], in1=E[:, 2:4, :], op=add)
                    nc.vector.tensor_tensor(out=S[:, 2:4, :], in0=E[:, 0:2, :], in1=E[:, 2:4, :], op=sub)
                    # s3 (bit2 = remaining slot bit b2): per group g=(j1m,j0),
                    # pair-tile R_g slots (j2) -> output blocks j1 = j2*4 + g
                    pieces = {  # g = j1m*2 + j0 -> (tile, half)
                        0: (O, 0), 1: (S, 0), 2: (O, 1), 3: (S, 1),
                    }
                    for g in range(4):
                        src_t, hf = pieces[g]
                        R = b_pool.tile([128, 2, FJ], mybir.dt.bfloat16, tag=f"R{g}", name=f"R{g}")
                        nc.vector.tensor_tensor(
                            out=R[:, 0, :], in0=src_t[:, 2 * hf, :], in1=src_t[:, 2 * hf + 1, :], op=add
                        )
                        nc.vector.tensor_tensor(
                            out=R[:, 1, :], in0=src_t[:, 2 * hf, :], in1=src_t[:, 2 * hf + 1, :], op=sub
                        )
                        # permutation add: vP host-ordered as (g, j2) pair groups
                        nc.vector.tensor_tensor(
                            out=R, in0=R, in1=vps[:, g * 8 : (g + 1) * 8, :], op=add
                        )
                        # store pair (j1=g, j1=g+4) via the Act HWDGE queue
                        nc.scalar.dma_start(out=o[:, h, g, :, :, :], in_=R)

            if loop_cm is not None:
                loop_cm.__exit__(None, None, None)
    nc.compile()
    return nc


def make_in_maps_h(value, src):
    vTb = np.ascontiguousarray(value.T).astype(BF16)  # [N, ROWS]
    vPb = vTb[src]  # host-permuted rows: vP[n] = vT[src[n]]
    Hs = np.ascontiguousarray(_hadamard_pm1(B) / 64.0).astype(BF16)
    in_maps = []
    for c in range(N_CORES):
        sl = slice(c * MPC, (c + 1) * MPC)
        # [N, MPC] -> [128, NH, KT, MH]: row t*128+p, col h*MH+m -> [p, h, t, m]
        vt = np.ascontiguousarray(
            vTb[:, sl].reshape(KT, 128, NH, MH).transpose(1, 2, 0, 3)
        )
        # vP ordered by (g, j2, j2s): block j1 = j2*4 + g
        vp = np.ascontiguousarray(
            vPb[:, sl]
            .reshape(2, 4, J2S, 128, NH, MH)      # [j2, g, j2s, p, h, m]
            .transpose(3, 4, 1, 0, 2, 5)          # [p, h, g, j2, j2s, m]
            .reshape(128, NH, NB, MH)
        )
        in_maps.append({"vT": vt, "vP": vp, "hb": Hs})
    return in_maps


def untile_out(o_tiled):
    """[128, NH, g:4, j2:2, J2S, MH] -> [N, MPC]; block j1 = j2*4 + g."""
    return np.ascontiguousarray(
        np.asarray(o_tiled).transpose(3, 2, 4, 0, 1, 5).reshape(N, MPC)
    )


# ---------------- dense fallback (arbitrary weight/permutation) ----------------


def build_dense():
    nc = bacc.Bacc("TRN2", target_bir_lowering=False)
    vT = nc.dram_tensor("vT", (N, MPC), mybir.dt.float32r, kind="ExternalInput")
    wgt = nc.dram_tensor("wgt", (N, N), mybir.dt.float32, kind="ExternalInput")
    prm = nc.dram_tensor("prm", (N, N), mybir.dt.float32, kind="ExternalInput")
    o = nc.dram_tensor("o", (N, MPC), mybir.dt.float32, kind="ExternalOutput")

    with tile.TileContext(nc) as tc:
        with (
            tc.tile_pool(name="vt", bufs=1) as vt_pool,
            tc.tile_pool(name="wp", bufs=2) as wp_pool,
            tc.tile_pool(name="pp", bufs=2) as pp_pool,
            tc.tile_pool(name="ps", bufs=4, space="PSUM") as ps_pool,
            tc.tile_pool(name="os", bufs=4) as os_pool,
        ):
            vts = []
            for t in range(KT):
                vt_t = vt_pool.tile([128, MPC], mybir.dt.float32r, tag=f"vt{t}")
                nc.sync.dma_start(out=vt_t, in_=vT[t * 128 : (t + 1) * 128, :])
                vts.append(vt_t)

            for nb in range(NB):
                n0 = nb * 128
                wp = wp_pool.tile([128, KT, 128], mybir.dt.float32r, tag="wp")
                pp = pp_pool.tile([128, KT, 128], mybir.dt.float32, tag="pp")
                wsrc = wgt[:, n0 : n0 + 128].rearrange("(kt p) j -> p kt j", p=128)
                psrc = prm[:, n0 : n0 + 128].rearrange("(kt p) j -> p kt j", p=128)
                nc.sync.dma_start(out=wp[:, :, :].bitcast(mybir.dt.float32), in_=wsrc)
                nc.sync.dma_start(out=pp, in_=psrc)
                nc.vector.tensor_tensor(
                    out=wp[:, :, :],
                    in0=wp[:, :, :].bitcast(mybir.dt.float32),
                    in1=pp[:, :, :],
                    op=mybir.AluOpType.add,
                )
                for mc in range(MC):
                    ps = ps_pool.tile([128, 512], mybir.dt.float32, tag="ps")
                    for kt in range(KT):
                        nc.tensor.matmul(
                            out=ps[:, :],
                            lhsT=wp[:, kt, :],
                            rhs=vts[kt][:, mc * 512 : (mc + 1) * 512],
                            start=(kt == 0),
                            stop=(kt == KT - 1),
                        )
                    ot = os_pool.tile([128, 512], mybir.dt.float32, tag="os")
                    nc.scalar.copy(out=ot[:, :], in_=ps[:, :])
                    nc.sync.dma_start(
                        out=o[n0 : n0 + 128, mc * 512 : (mc + 1) * 512], in_=ot
                    )
    nc.compile()
    return nc


def make_in_maps(value, weight, permutation):
    vT = np.ascontiguousarray(value.T)  # [N, ROWS]
    w = np.ascontiguousarray(weight, dtype=np.float32)
    p = np.ascontiguousarray(permutation, dtype=np.float32)
    in_maps = []
    for c in range(N_CORES):
        in_maps.append(
            {
                "vT": np.ascontiguousarray(vT[:, c * MPC : (c + 1) * MPC]),
                "wgt": w,
                "prm": p,
            }
        )
    return in_maps


def kernel(value, weight, permutation):
    value = np.asarray(value, dtype=np.float32)
    weight = np.asarray(weight, dtype=np.float32)
    permutation = np.asarray(permutation, dtype=np.float32)
    src = check_structure(weight, permutation)
    if src is not None:
        if "had" not in _cache:
            _cache["had"] = build_hadamard()
        nc = _cache["had"]
        in_maps = make_in_maps_h(value, src)
        res = run_bass_kernel_spmd(nc, in_maps, core_ids=list(range(N_CORES)))
        out = np.concatenate(
            [
                untile_out(res.results[c]["o"]).T.astype(np.float32)
                for c in range(N_CORES)
            ],
            axis=0,
        )
        return out
    if "dense" not in _cache:
        _cache["dense"] = build_dense()
    nc = _cache["dense"]
    in_maps = make_in_maps(value, weight, permutation)
    res = run_bass_kernel_spmd(nc, in_maps, core_ids=list(range(N_CORES)))
    out = np.concatenate(
        [np.ascontiguousarray(res.results[c]["o"].T) for c in range(N_CORES)], axis=0
    )
    return out
